# revision 32
# baseline (speedup 1.0000x reference)
"""Causal attention kernel for 8 Trainium2 NeuronCores.

Problem: x[4, 4096, 512] @ {Wq,Wk,Wv}[512, 128] -> causal attention -> [4, 4096, 128].

Sharding: 2 cores per batch, interleaved over KEY chunks. Core c = 2b+p
(batch b, parity p) owns key chunks {2i+p : i=0..15} (chunks of 128 keys),
and computes, for every query block of its batch, the partial softmax
numerator and denominator over its keys. The host sums the two partials and
divides. Causality makes query block qg (512 rows) attend key chunks
0..4qg+3, of which each parity owns exactly 2qg+2 -> both cores run the
identical program (exact load balance); only the last two local chunks of
each block need a (parity-dependent, input-supplied) additive mask.

On-device layout: scores are computed transposed, S^T[key, q]:
  - K^T[d, key], Q^T[d, q] come from host-pre-transposed x (no on-device
    transposes anywhere); 1/sqrt(d) is folded into Wq on the host
  - S^T chunk = matmul(lhsT=K^T[:, chunk], rhs=Q^T[:, qblock])  [N=512]
  - E = exp(S^T + mask) via ScalarE, straight out of PSUM
  - numerator^T[e, q] accumulates in PSUM: matmul(lhsT=V[chunk], rhs=E)
  - denominator[1, q] accumulates in PSUM: matmul(lhsT=ones, rhs=E)

MODE selects matmul operand precision (PSUM accumulation is always fp32):
  "bf16" - operands bf16 (x/W cast on host, halving input DMA); fastest
  "f32r" - single-pass fp32 matmul, ~12-bit mantissa operands
  "f32"  - exact fp32 (2-pass LOW_HIGH matmuls), slowest
"""

import math

import numpy as np

B, S, DIN, DOUT = 4, 4096, 512, 128
NCORES = 8
TQ = 512            # query block size
NQB = S // TQ       # 8 query blocks per batch
KC = 128            # key chunk size
NKLOC = S // KC // 2  # 16 key chunks owned per core
NEG = -1.0e9

MODE = "bf16"

_cache = {}


def _np_in_dtype(mode):
    if mode == "bf16":
        import ml_dtypes

        return ml_dtypes.bfloat16
    return np.float32


def _build_nc(mode=None):
    import concourse.bacc as bacc
    import concourse.mybir as mybir
    import concourse.tile as tile

    mode = MODE if mode is None else mode
    f32 = mybir.dt.float32
    mdt = {
        "f32": f32,
        "f32r": mybir.dt.float32r,
        "bf16": mybir.dt.bfloat16,
    }[mode]

    nc = bacc.Bacc(None, target_bir_lowering=False, debug=False)

    NDC = DIN // 128    # 4 contraction chunks for the projections
    SK = NKLOC * KC     # 2048 owned keys
    WMC = 3 * NDC * DOUT + 2 * TQ  # packed w+masks columns

    # All inputs are host-packed partition-major so every DMA is contiguous:
    # xb[p, c, s] = x[b].T[c*128+p, s], wm[p, :] = [wq|wk|wv chunks, masks]
    xb_d = nc.declare_dram_parameter("xb", [128, NDC, S], mdt, isOutput=False)
    xk_d = nc.declare_dram_parameter("xk", [128, NDC, SK], mdt, isOutput=False)
    wm_d = nc.declare_dram_parameter("wm", [128, WMC], mdt, isOutput=False)
    numT = nc.declare_dram_parameter("numT", [DOUT, S], f32, isOutput=True)
    den = nc.declare_dram_parameter("den", [NQB, TQ], f32, isOutput=True)

    import concourse.bass_isa as bass_isa

    with tile.TileContext(nc) as tc:
        with (
            tc.tile_pool(name="persist", bufs=1) as persist,
            tc.tile_pool(name="pss", bufs=4, space="PSUM") as pss,
            tc.tile_pool(name="pso", bufs=2, space="PSUM") as pso,
            tc.tile_pool(name="psd", bufs=2, space="PSUM") as psd,
            tc.tile_pool(name="etile", bufs=6) as etile,
            tc.tile_pool(name="otile", bufs=2) as otile,
            tc.tile_pool(name="dtile", bufs=2) as dtile,
        ):
            # ---- resident SBUF tensors ----
            xb_t = persist.tile([128, NDC, S], mdt, tag="xb")
            xk_t = persist.tile([128, NDC, SK], mdt, tag="xk")
            wm_t = persist.tile([128, WMC], mdt, tag="wm")
            qT = persist.tile([128, S], mdt, tag="qT")
            kT = persist.tile([128, SK], mdt, tag="kT")
            v_t = persist.tile([128, NKLOC, DOUT], mdt, tag="v")

            def w_ap(wi, c):
                return wm_t[:, (wi * NDC + c) * DOUT:(wi * NDC + c + 1) * DOUT]

            def mask_ap(m):
                return wm_t[:, 3 * NDC * DOUT + m * TQ:3 * NDC * DOUT + (m + 1) * TQ]

            # Input DMA: issue order matters (each HWDGE ring is FIFO and a
            # dma_start occupies the ring ~0.6us regardless of size), so use
            # few, large DMAs, most-urgent first. sync ring: K/V-path inputs;
            # scalar ring: Q-path inputs. rearrange folds the DIN chunking
            # into a single access pattern.
            ones_f = persist.tile([128, 1], f32, tag="ones_f")
            nc.vector.memset(ones_f, 1.0)
            ones = persist.tile([128, 1], mdt, tag="ones")
            nc.vector.tensor_copy(ones[:], ones_f[:])

            nc.sync.dma_start(out=wm_t[:], in_=wm_d[:])
            # xk: small lead piece so K-proj slice 0 starts early, then halves
            for sl in (slice(0, 512), slice(512, SK // 2), slice(SK // 2, SK)):
                nc.sync.dma_start(out=xk_t[:, :, sl], in_=xk_d[:, :, sl])
            # xb on the scalar ring, progressive for Q-proj
            for sl in (
                slice(0, 512),
                slice(512, 1024),
                slice(1024, 2048),
                slice(2048, 3072),
                slice(3072, S),
            ):
                nc.scalar.dma_start(out=xb_t[:, :, sl], in_=xb_d[:, :, sl])

            # ---- projections (K/V first: attention consumes them earliest) ----
            for s512 in range(SK // 512):  # K^T over owned keys
                ps = pss.tile([128, 512], f32, tag="ps_s", name=f"psk{s512}")
                for c in range(NDC):
                    nc.tensor.matmul(
                        ps[:],
                        w_ap(1, c),
                        xk_t[:, c, s512 * 512:(s512 + 1) * 512],
                        start=(c == 0),
                        stop=(c == NDC - 1),
                    )
                nc.vector.tensor_copy(kT[:, s512 * 512:(s512 + 1) * 512], ps[:])
            for t in range(NKLOC):  # V[key, e] natural layout, owned keys
                ps = pss.tile([128, 512], f32, tag="ps_s", name=f"psv{t}")
                for c in range(NDC):
                    nc.tensor.matmul(
                        ps[:, :DOUT],
                        xk_t[:, c, t * KC:(t + 1) * KC],
                        w_ap(2, c),
                        start=(c == 0),
                        stop=(c == NDC - 1),
                    )
                nc.vector.tensor_copy(v_t[:, t, :], ps[:, :DOUT])
            for s512 in range(S // 512):  # Q^T over all queries, in block order
                ps = pss.tile([128, 512], f32, tag="ps_s", name=f"psq{s512}")
                for c in range(NDC):
                    nc.tensor.matmul(
                        ps[:],
                        w_ap(0, c),
                        xb_t[:, c, s512 * 512:(s512 + 1) * 512],
                        start=(c == 0),
                        stop=(c == NDC - 1),
                    )
                nc.vector.tensor_copy(qT[:, s512 * 512:(s512 + 1) * 512], ps[:])

            # ---- attention ----
            for qg in range(NQB):
                n_loc = 2 * qg + 2
                po = pso.tile([128, TQ], f32, tag="po", name=f"po{qg}")
                pd = psd.tile([1, TQ], f32, tag="pd", name=f"pd{qg}")
                for i in range(n_loc):
                    ps = pss.tile([128, TQ], f32, tag="ps_s", name=f"pss{qg}_{i}")
                    masked = i >= n_loc - 2
                    if masked:
                        # pre-bias PSUM with the additive causal mask (off the
                        # critical path), then accumulate scores onto it
                        nc.vector.tensor_copy(ps[:], mask_ap(i - (n_loc - 2)))
                    nc.tensor.matmul(
                        ps[:],
                        kT[:, i * KC:(i + 1) * KC],
                        qT[:, qg * TQ:(qg + 1) * TQ],
                        start=not masked,
                        stop=True,
                    )
                    e = etile.tile([128, TQ], mdt, tag="e", name=f"e{qg}_{i}")
                    nc.scalar.activation(
                        e[:], ps[:], mybir.ActivationFunctionType.Exp
                    )
                    nc.tensor.matmul(
                        po[:],
                        v_t[:, i, :],
                        e[:],
                        start=(i == 0),
                        stop=(i == n_loc - 1),
                    )
                    nc.tensor.matmul(
                        pd[:],
                        ones[:],
                        e[:],
                        start=(i == 0),
                        stop=(i == n_loc - 1),
                    )
                o = otile.tile([128, TQ], f32, tag="o", name=f"o{qg}")
                nc.vector.tensor_copy(o[:], po[:])
                nc.scalar.dma_start(out=numT[:, qg * TQ:(qg + 1) * TQ], in_=o[:])
                d = dtile.tile([1, TQ], f32, tag="d", name=f"d{qg}")
                nc.vector.tensor_copy(d[:], pd[:])
                nc.sync.dma_start(out=den[qg:qg + 1, :], in_=d[:])

    nc.finalize()
    return nc


def _owned_keys(par):
    return np.concatenate(
        [np.arange((2 * i + par) * KC, (2 * i + par) * KC + KC) for i in range(NKLOC)]
    )


def _build_masks(par):
    # last two local chunks of each query block: relative chunk r0 = par,
    # r1 = 2 + par; element [k, q] allowed iff 128*r + k <= q
    r = np.array([par, 2 + par])[:, None, None]
    k = np.arange(KC)[None, :, None]
    q = np.arange(TQ)[None, None, :]
    allowed = (KC * r + k) <= q
    return np.where(allowed, np.float32(0.0), np.float32(NEG)).astype(np.float32)


def _get_nc():
    if "nc" not in _cache:
        _cache["nc"] = _build_nc()
    return _cache["nc"]


def _pack_pm(a):
    # [DIN, cols] -> partition-major [128, DIN//128, cols]
    return np.ascontiguousarray(a.reshape(DIN // 128, 128, a.shape[1]).transpose(1, 0, 2))


def _prepare_in_maps(x, Wq, Wk, Wv, mode=None):
    mode = MODE if mode is None else mode
    idt = _np_in_dtype(mode)
    ws = [(Wq / math.sqrt(DOUT)).astype(idt), Wk.astype(idt), Wv.astype(idt)]
    w_pack = np.concatenate(
        [_pack_pm(w).reshape(128, -1) for w in ws], axis=1
    )  # [128, 1536]
    in_maps = []
    for c in range(NCORES):
        b, par = c // 2, c % 2
        xbt = x[b].T.astype(idt)
        m = _build_masks(par).astype(idt)  # [2, 128, 512]
        wm = np.concatenate(
            [w_pack, np.ascontiguousarray(m.transpose(1, 0, 2)).reshape(128, -1)],
            axis=1,
        )
        in_maps.append({
            "xb": _pack_pm(xbt),
            "xk": _pack_pm(np.ascontiguousarray(xbt[:, _owned_keys(par)])),
            "wm": np.ascontiguousarray(wm),
        })
    return in_maps


def _gather(results):
    out = np.empty((B, S, DOUT), dtype=np.float32)
    for b in range(B):
        r0, r1 = results[2 * b], results[2 * b + 1]
        num = r0["numT"].astype(np.float64).T + r1["numT"].astype(np.float64).T
        d = r0["den"].astype(np.float64).reshape(-1) + r1["den"].astype(
            np.float64
        ).reshape(-1)
        out[b] = (num / d[:, None]).astype(np.float32)
    return out


def kernel(**inputs):
    from concourse.bass_utils import run_bass_kernel_spmd

    x = np.asarray(inputs["x"], dtype=np.float32)
    Wq = np.asarray(inputs["Wq"], dtype=np.float32)
    Wk = np.asarray(inputs["Wk"], dtype=np.float32)
    Wv = np.asarray(inputs["Wv"], dtype=np.float32)

    nc = _get_nc()
    in_maps = _prepare_in_maps(x, Wq, Wk, Wv)
    res = run_bass_kernel_spmd(nc, in_maps, list(range(NCORES)))
    return _gather(res.results)
